# revision 16
# baseline (speedup 1.0000x reference)
"""Trainium2 Bass kernel for the attention module:

    keys   = einsum('sbd,ed->bse', encoder_out, W_enc)
    query  = decoder_hidden @ W_dec.T
    scores = tanh(keys + query)
    logits = einsum('bse,e->bs', scores, V_att[0])
    alpha  = softmax(logits, axis=1)
    context= einsum('bs,sbd->bd', alpha, encoder_out)
    returns (context[:, None, :], alpha.T[None])

Sharding: data-parallel over batch across 8 NeuronCores (4 batch rows per
core); the three weight tensors are replicated.  Each core runs an
identical program on its batch shard; no collectives are needed.

Per-core dataflow (b = batch row, iterating over seq in 512-wide tiles):
  1. DMA encoder_out rows naturally: e_nat tiles (128 seq-part, 1024 dim).
  2. PE-transpose 128x128 blocks -> e_t tiles (128 dim-part, 512 seq).
  3. Phase A: scores^T[e,s] psum = sum_dc W_encT[dc,e-chunk].T @ e_t[dc],
     f32r matmuls (full rate), accumulated over 8 contraction chunks.
  4. tanh via ACT with per-partition bias = q[e] chunk (query folded in).
  5. logits(1,512) psum = sum_ec V[ec].T @ scoresT[ec]  (f32r matmuls).
  6. exp via ACT (logits are bounded by sum|V| ~ 25, so no max-subtraction
     is needed) + fused partial denominator via accum_out.
  7. w row broadcast across partitions with a 0-stride SBUF->SBUF DMA;
     phase C partial context via DVE tensor_tensor_reduce with the e_t
     tiles still resident: ctx[dc] col += sum_s e_t[dc][d,s] * w[s].
  8. Per-b epilogue: denom reduce + reciprocal, alpha = w * (1/denom),
     context = (sum of partials) * (1/denom), PE-transpose to row layout,
     DMA out.
"""

import sys

import numpy as np

_REPO = "/opt/trn_rl_repo"
if _REPO not in sys.path:
    sys.path.insert(0, _REPO)

import concourse.bacc as bacc
import concourse.bass as bass
import concourse.mybir as mybir
import concourse.tile as tile
from concourse.bass import MemorySpace
from concourse.masks import make_identity

F32 = mybir.dt.float32
F32R = mybir.dt.float32r
AF = mybir.ActivationFunctionType
ALU = mybir.AluOpType
AXIS = mybir.AxisListType
P = 128

SEQ, BATCH, DIM = 2048, 32, 1024
N_CORES = 8
BPC = BATCH // N_CORES  # batch rows per core


def build_attention_nc(seq=SEQ, bpc=BPC, dim=DIM, mm_dtype=F32R, reps=1):
    """Build the single-core SPMD Bass program (same program on all cores)."""
    SW = 512              # seq tile width (free dim) for the main loop
    SS = SW // P          # 128-row subtiles per seq tile
    NIT = seq // SW       # seq tiles per batch row
    DC = dim // P         # contraction (d) chunks
    EC = dim // P         # output (e) chunks
    assert seq % SW == 0 and dim % P == 0

    nc = bacc.Bacc("TRN2", target_bir_lowering=False, debug=False,
                   num_devices=N_CORES)

    enc = nc.dram_tensor("encoder_out", (seq, bpc, dim), F32,
                         kind="ExternalInput").ap()
    dh = nc.dram_tensor("decoder_hidden", (bpc, 1, dim), F32,
                        kind="ExternalInput").ap()
    w_enc = nc.dram_tensor("W_enc", (dim, dim), F32, kind="ExternalInput").ap()
    w_dec = nc.dram_tensor("W_dec", (dim, dim), F32, kind="ExternalInput").ap()
    v_att = nc.dram_tensor("V_att", (1, dim), F32, kind="ExternalInput").ap()
    ctx_out = nc.dram_tensor("context", (bpc, 1, dim), F32,
                             kind="ExternalOutput").ap()
    alpha_out = nc.dram_tensor("alpha", (1, bpc, seq), F32,
                               kind="ExternalOutput").ap()
    # DRAM scratch used to broadcast per-row scalars across partitions
    # (SBUF-source DMAs cannot have a 0-stride partition dim; DRAM can).
    w_scr = nc.dram_tensor("w_scr", (bpc, seq), F32).ap()
    rden_scr = nc.dram_tensor("rden_scr", (bpc, 1), F32).ap()

    with tile.TileContext(nc) as tc:
        with (
            tc.tile_pool(name="consts", bufs=1) as consts,
            tc.tile_pool(name="psum", bufs=2, space=MemorySpace.PSUM) as psum,
        ):
            identity = consts.tile([P, P], F32)
            make_identity(nc, identity[:])

            # Persistent small tensors.
            q_sb = [consts.tile([P, bpc], F32, tag=f"q{ec}", name=f"q{ec}") for ec in range(EC)]
            v_t = [consts.tile([P, 1], mm_dtype, tag=f"v{ec}", name=f"v{ec}") for ec in range(EC)]
            w_t = [consts.tile([P, dim], mm_dtype, tag=f"wt{dc}", name=f"wt{dc}") for dc in range(DC)]
            # Per-b row state lives on partition 0 (compute ops cannot start
            # at partition bases other than 0/32/64/96).
            w_row = [consts.tile([1, seq], F32, tag=f"w_row{b}",
                                 name=f"w_row{b}") for b in range(bpc)]
            dpart = [consts.tile([1, NIT], F32, tag=f"dpart{b}",
                                 name=f"dpart{b}") for b in range(bpc)]
            den = consts.tile([1, bpc], F32, tag="den")
            rden = consts.tile([1, bpc], F32, tag="rden")
            ctx_cols = [consts.tile([P, bpc * NIT], F32, tag=f"cc{dc}", name=f"cc{dc}")
                        for dc in range(DC)]

            # ---------------- setup: W_dec -> q, W_enc -> w_t, V -> v_t ----
            with tc.tile_pool(name="setup", bufs=1) as setup:
                # decoder_hidden, transposed to (dim-part, bpc).
                dh_nat = setup.tile([bpc, dim], F32, tag="dh_nat")
                nc.sync.dma_start(dh_nat[:], dh[0:bpc, 0, :])
                dht = [setup.tile([P, bpc], F32, tag=f"dht{dc}", name=f"dht{dc}")
                       for dc in range(DC)]
                for dc in range(DC):
                    pt = psum.tile([P, bpc], F32, tag="l")
                    nc.tensor.transpose(pt[:], dh_nat[:, dc * P:(dc + 1) * P],
                                        identity[:bpc, :bpc])
                    nc.vector.tensor_copy(dht[dc][:], pt[:])

                # W_dec: load natural (e-part, d), transpose to (d-part, e).
                wdec_t = [setup.tile([P, dim], F32, tag=f"wdt{dc}", name=f"wdt{dc}")
                          for dc in range(DC)]
                for ec in range(EC):
                    wnat = setup.tile([P, dim], F32, tag="wnat")
                    nc.sync.dma_start(wnat[:], w_dec[ec * P:(ec + 1) * P, :])
                    for dc in range(DC):
                        ptw = psum.tile([P, P], F32, tag="tp")
                        nc.tensor.transpose(ptw[:], wnat[:, dc * P:(dc + 1) * P],
                                            identity[:])
                        nc.vector.tensor_copy(
                            wdec_t[dc][:, ec * P:(ec + 1) * P], ptw[:])

                # q[e, b] = sum_d W_dec[e, d] * dh[b, d]  (exact fp32 matmuls)
                for ec in range(EC):
                    pq = psum.tile([P, bpc], F32, tag="l")
                    for dc in range(DC):
                        nc.tensor.matmul(pq[:],
                                         wdec_t[dc][:, ec * P:(ec + 1) * P],
                                         dht[dc][:],
                                         start=(dc == 0), stop=(dc == DC - 1))
                    nc.vector.tensor_copy(q_sb[ec][:], pq[:])

                # W_enc: load natural, transpose into persistent w_t.
                for ec in range(EC):
                    wnat = setup.tile([P, dim], F32, tag="wnat")
                    nc.sync.dma_start(wnat[:], w_enc[ec * P:(ec + 1) * P, :])
                    for dc in range(DC):
                        ptw = psum.tile([P, P], F32, tag="tp")
                        nc.tensor.transpose(ptw[:], wnat[:, dc * P:(dc + 1) * P],
                                            identity[:])
                        nc.vector.tensor_copy(
                            w_t[dc][:, ec * P:(ec + 1) * P], ptw[:])

                # V_att chunks -> (128, 1) columns.
                v_nat = setup.tile([1, dim], F32, tag="v_nat")
                nc.sync.dma_start(v_nat[:], v_att[0:1, :])
                for ec in range(EC):
                    pv = psum.tile([P, 1], F32, tag="l")
                    nc.tensor.transpose(pv[:], v_nat[:, ec * P:(ec + 1) * P],
                                        identity[:1, :1])
                    nc.vector.tensor_copy(v_t[ec][:], pv[:])

            # ---------------- main loop ----------------
            with tc.tile_pool(name="main", bufs=2) as main:
              for _rep in range(reps):
                for b in range(bpc):
                    for it in range(NIT):
                        s0 = it * SW
                        # 1. natural loads (128 descriptors x 4KB each)
                        e_nat = []
                        for ss in range(SS):
                            t = main.tile([P, dim], F32, tag=f"enat{ss}", name=f"enat{ss}")
                            nc.sync.dma_start(
                                t[:], enc[s0 + ss * P:s0 + (ss + 1) * P, b, :])
                            e_nat.append(t)
                        # 2. transpose to (d-part, s)
                        e_t = []
                        for dc in range(DC):
                            pt = psum.tile([P, SW], F32, tag="tp")
                            for ss in range(SS):
                                nc.tensor.transpose(
                                    pt[:, ss * P:(ss + 1) * P],
                                    e_nat[ss][:, dc * P:(dc + 1) * P],
                                    identity[:])
                            t = main.tile([P, SW], mm_dtype, tag=f"et{dc}", name=f"et{dc}")
                            nc.vector.tensor_copy(t[:], pt[:])
                            e_t.append(t)
                        # 3+4. phase A matmuls + tanh(.. + q)
                        sc = []
                        for ec in range(EC):
                            pk = psum.tile([P, SW], F32, tag="mm")
                            for dc in range(DC):
                                nc.tensor.matmul(
                                    pk[:],
                                    w_t[dc][:, ec * P:(ec + 1) * P],
                                    e_t[dc][:],
                                    start=(dc == 0), stop=(dc == DC - 1))
                            t = main.tile([P, SW], mm_dtype, tag=f"sc{ec}", name=f"sc{ec}")
                            nc.scalar.activation(t[:], pk[:], AF.Tanh,
                                                 bias=q_sb[ec][:, b:b + 1])
                            sc.append(t)
                        # 5. logits
                        pl = psum.tile([1, SW], F32, tag="l")
                        for ec in range(EC):
                            nc.tensor.matmul(pl[:], v_t[ec][:],
                                             sc[ec][:],
                                             start=(ec == 0),
                                             stop=(ec == EC - 1))
                        # 6. exp + partial denominator
                        nc.scalar.activation(
                            w_row[b][0:1, s0:s0 + SW], pl[:], AF.Exp,
                            accum_out=dpart[b][0:1, it:it + 1])
                        # 7. broadcast w across partitions via a DRAM
                        #    round-trip (0-stride DRAM-source DMA), then
                        #    accumulate context partials on DVE.
                        nc.sync.dma_start(w_scr[b:b + 1, s0:s0 + SW],
                                          w_row[b][0:1, s0:s0 + SW])
                        w_bc = main.tile([P, SW], F32, tag="w_bc")
                        nc.sync.dma_start(
                            w_bc[:],
                            w_scr[b:b + 1, s0:s0 + SW].to_broadcast((P, SW)))
                        col = b * NIT + it
                        for dc in range(DC):
                            scratch = main.tile([P, SW], F32, tag="ttr_scr")
                            nc.vector.scalar_tensor_tensor(
                                out=scratch[:], in0=e_t[dc][:].bitcast(F32),
                                scalar=1.0, in1=w_bc[:],
                                op0=ALU.mult, op1=ALU.mult,
                                accum_out=ctx_cols[dc][:, col:col + 1])

                    # ------------ per-b epilogue ------------
                    nc.vector.tensor_reduce(den[0:1, b:b + 1], dpart[b][:],
                                            axis=AXIS.X, op=ALU.add)
                    nc.vector.reciprocal(rden[0:1, b:b + 1], den[0:1, b:b + 1])
                    # normalize alpha in place: w_row[b] *= 1/denom
                    nc.vector.tensor_scalar_mul(w_row[b][:], w_row[b][:],
                                                rden[0:1, b:b + 1])
                    nc.sync.dma_start(alpha_out[0:1, b, :], w_row[b][:])

                    ctxall = main.tile([P, DC], F32, tag="ctxall")
                    for dc in range(DC):
                        nc.vector.tensor_reduce(
                            ctxall[:, dc:dc + 1],
                            ctx_cols[dc][:, b * NIT:(b + 1) * NIT],
                            axis=AXIS.X, op=ALU.add)
                    nc.sync.dma_start(rden_scr[b:b + 1, :],
                                      rden[0:1, b:b + 1])
                    rden_b = main.tile([P, 1], F32, tag="rden_b")
                    nc.sync.dma_start(rden_b[:],
                                      rden_scr[b:b + 1, :].to_broadcast((P, 1)))
                    ctxs = main.tile([P, DC], F32, tag="ctxs")
                    nc.vector.tensor_scalar_mul(ctxs[:], ctxall[:], rden_b[:])
                    pc = psum.tile([DC, P], F32, tag="l")
                    nc.tensor.transpose(pc[:], ctxs[:], identity[:])
                    ctx_t = main.tile([DC, P], F32, tag="ctx_t")
                    nc.vector.tensor_copy(ctx_t[:], pc[:])
                    nc.sync.dma_start(ctx_out[b, 0, :], ctx_t[:])

    nc.compile()
    return nc


_NC_CACHE = {}


def _get_nc():
    key = (SEQ, BPC, DIM)
    if key not in _NC_CACHE:
        _NC_CACHE[key] = build_attention_nc()
    return _NC_CACHE[key]


def _make_in_maps(decoder_hidden, encoder_out, W_enc, W_dec, V_att):
    decoder_hidden = np.ascontiguousarray(
        np.asarray(decoder_hidden, dtype=np.float32))
    encoder_out = np.asarray(encoder_out, dtype=np.float32)
    W_enc = np.ascontiguousarray(np.asarray(W_enc, dtype=np.float32))
    W_dec = np.ascontiguousarray(np.asarray(W_dec, dtype=np.float32))
    V_att = np.ascontiguousarray(np.asarray(V_att, dtype=np.float32))
    in_maps = []
    for c in range(N_CORES):
        bs = slice(c * BPC, (c + 1) * BPC)
        in_maps.append({
            "encoder_out": np.ascontiguousarray(encoder_out[:, bs, :]),
            "decoder_hidden": decoder_hidden[bs],
            "W_enc": W_enc,
            "W_dec": W_dec,
            "V_att": V_att,
        })
    return in_maps


def run_on_hw(decoder_hidden, encoder_out, W_enc, W_dec, V_att, **bench_kwargs):
    """Run on the 8 NeuronCores; returns (BassKernelResults, context, alpha)."""
    from concourse.bass_utils import run_bass_kernel_spmd

    nc = _get_nc()
    in_maps = _make_in_maps(decoder_hidden, encoder_out, W_enc, W_dec, V_att)
    res = run_bass_kernel_spmd(nc, in_maps, list(range(N_CORES)),
                               **bench_kwargs)
    context = np.concatenate([res.results[c]["context"]
                              for c in range(N_CORES)], axis=0)
    alpha = np.concatenate([res.results[c]["alpha"]
                            for c in range(N_CORES)], axis=1)
    return res, context, alpha


def kernel(decoder_hidden, encoder_out, W_enc, W_dec, V_att):
    _, context, alpha = run_on_hw(decoder_hidden, encoder_out, W_enc, W_dec,
                                  V_att)
    return context, alpha


# revision 17
# speedup vs baseline: 1.7313x; 1.7313x over previous
"""Trainium2 Bass kernel for the attention module:

    keys   = einsum('sbd,ed->bse', encoder_out, W_enc)
    query  = decoder_hidden @ W_dec.T
    scores = tanh(keys + query)
    logits = einsum('bse,e->bs', scores, V_att[0])
    alpha  = softmax(logits, axis=1)
    context= einsum('bs,sbd->bd', alpha, encoder_out)
    returns (context[:, None, :], alpha.T[None])

Sharding: data-parallel over batch across 8 NeuronCores (4 batch rows per
core); the three weight tensors are replicated.  Each core runs an
identical program on its batch shard; no collectives are needed.

Per-core dataflow (b = batch row, iterating over seq in 512-wide tiles):
  1. DMA encoder_out rows naturally: e_nat tiles (128 seq-part, 1024 dim).
  2. PE-transpose 128x128 blocks -> e_t tiles (128 dim-part, 512 seq).
  3. Phase A: scores^T[e,s] psum = sum_dc W_encT[dc,e-chunk].T @ e_t[dc],
     f32r matmuls (full rate), accumulated over 8 contraction chunks.
  4. tanh via ACT with per-partition bias = q[e] chunk (query folded in).
  5. logits(1,512) psum = sum_ec V[ec].T @ scoresT[ec]  (f32r matmuls).
  6. exp via ACT (logits are bounded by sum|V| ~ 25, so no max-subtraction
     is needed) + fused partial denominator via accum_out.
  7. w row broadcast across partitions with a 0-stride SBUF->SBUF DMA;
     phase C partial context via DVE tensor_tensor_reduce with the e_t
     tiles still resident: ctx[dc] col += sum_s e_t[dc][d,s] * w[s].
  8. Per-b epilogue: denom reduce + reciprocal, alpha = w * (1/denom),
     context = (sum of partials) * (1/denom), PE-transpose to row layout,
     DMA out.
"""

import sys

import numpy as np

_REPO = "/opt/trn_rl_repo"
if _REPO not in sys.path:
    sys.path.insert(0, _REPO)

import concourse.bacc as bacc
import concourse.bass as bass
import concourse.mybir as mybir
import concourse.tile as tile
from concourse.bass import MemorySpace
from concourse.masks import make_identity

F32 = mybir.dt.float32
F32R = mybir.dt.float32r
BF16 = mybir.dt.bfloat16
AF = mybir.ActivationFunctionType
ALU = mybir.AluOpType
AXIS = mybir.AxisListType
P = 128

SEQ, BATCH, DIM = 2048, 32, 1024
N_CORES = 8
BPC = BATCH // N_CORES  # batch rows per core


def build_attention_nc(seq=SEQ, bpc=BPC, dim=DIM, mm_dtype=F32R, reps=1,
                       e_dtype=None):
    """Build the single-core SPMD Bass program (same program on all cores)."""
    SW = 512              # seq tile width (free dim) for the main loop
    SS = SW // P          # 128-row subtiles per seq tile
    NIT = seq // SW       # seq tiles per batch row
    DC = dim // P         # contraction (d) chunks
    EC = dim // P         # output (e) chunks
    assert seq % SW == 0 and dim % P == 0
    # e_dtype=BF16 moves the E transposes from the PE (fp32 matmul-transpose)
    # to the DMA engines (XBAR transpose, 16-bit only); phase A then runs in
    # bf16 and phase C reads bf16 E values.
    use_bf16_e = e_dtype is not None and e_dtype == BF16
    wt_dtype = BF16 if use_bf16_e else mm_dtype

    nc = bacc.Bacc("TRN2", target_bir_lowering=False, debug=False,
                   num_devices=N_CORES)

    enc = nc.dram_tensor("encoder_out", (seq, bpc, dim), F32,
                         kind="ExternalInput").ap()
    dh = nc.dram_tensor("decoder_hidden", (bpc, 1, dim), F32,
                        kind="ExternalInput").ap()
    w_enc = nc.dram_tensor("W_enc", (dim, dim), F32, kind="ExternalInput").ap()
    w_dec = nc.dram_tensor("W_dec", (dim, dim), F32, kind="ExternalInput").ap()
    v_att = nc.dram_tensor("V_att", (1, dim), F32, kind="ExternalInput").ap()
    ctx_out = nc.dram_tensor("context", (bpc, 1, dim), F32,
                             kind="ExternalOutput").ap()
    alpha_out = nc.dram_tensor("alpha", (1, bpc, seq), F32,
                               kind="ExternalOutput").ap()
    # DRAM scratch used to broadcast per-row scalars across partitions
    # (SBUF-source DMAs cannot have a 0-stride partition dim; DRAM can).
    w_scr = nc.dram_tensor("w_scr", (bpc, seq), F32).ap()
    rden_scr = nc.dram_tensor("rden_scr", (bpc, 1), F32).ap()

    with tile.TileContext(nc) as tc:
        with (
            tc.tile_pool(name="consts", bufs=1) as consts,
            tc.tile_pool(name="psum", bufs=2, space=MemorySpace.PSUM) as psum,
        ):
            identity = consts.tile([P, P], F32)
            make_identity(nc, identity[:])

            # Persistent small tensors.
            q_sb = [consts.tile([P, bpc], F32, tag=f"q{ec}", name=f"q{ec}") for ec in range(EC)]
            v_t = [consts.tile([P, 1], mm_dtype, tag=f"v{ec}", name=f"v{ec}") for ec in range(EC)]
            w_t = [consts.tile([P, dim], wt_dtype, tag=f"wt{dc}", name=f"wt{dc}") for dc in range(DC)]
            # Per-b row state lives on partition 0 (compute ops cannot start
            # at partition bases other than 0/32/64/96).
            w_row = [consts.tile([1, seq], F32, tag=f"w_row{b}",
                                 name=f"w_row{b}") for b in range(bpc)]
            dpart = [consts.tile([1, NIT], F32, tag=f"dpart{b}",
                                 name=f"dpart{b}") for b in range(bpc)]
            den = consts.tile([1, bpc], F32, tag="den")
            rden = consts.tile([1, bpc], F32, tag="rden")
            ctx_cols = [consts.tile([P, bpc * NIT], F32, tag=f"cc{dc}", name=f"cc{dc}")
                        for dc in range(DC)]

            # ---------------- setup: W_dec -> q, W_enc -> w_t, V -> v_t ----
            with tc.tile_pool(name="setup", bufs=1) as setup:
                # decoder_hidden, transposed to (dim-part, bpc).
                dh_nat = setup.tile([bpc, dim], F32, tag="dh_nat")
                nc.sync.dma_start(dh_nat[:], dh[0:bpc, 0, :])
                dht = [setup.tile([P, bpc], F32, tag=f"dht{dc}", name=f"dht{dc}")
                       for dc in range(DC)]
                for dc in range(DC):
                    pt = psum.tile([P, bpc], F32, tag="l")
                    nc.tensor.transpose(pt[:], dh_nat[:, dc * P:(dc + 1) * P],
                                        identity[:bpc, :bpc])
                    nc.vector.tensor_copy(dht[dc][:], pt[:])

                # W_dec: load natural (e-part, d), transpose to (d-part, e).
                wdec_t = [setup.tile([P, dim], F32, tag=f"wdt{dc}", name=f"wdt{dc}")
                          for dc in range(DC)]
                for ec in range(EC):
                    wnat = setup.tile([P, dim], F32, tag="wnat")
                    nc.sync.dma_start(wnat[:], w_dec[ec * P:(ec + 1) * P, :])
                    for dc in range(DC):
                        ptw = psum.tile([P, P], F32, tag="tp")
                        nc.tensor.transpose(ptw[:], wnat[:, dc * P:(dc + 1) * P],
                                            identity[:])
                        nc.vector.tensor_copy(
                            wdec_t[dc][:, ec * P:(ec + 1) * P], ptw[:])

                # q[e, b] = sum_d W_dec[e, d] * dh[b, d]  (exact fp32 matmuls)
                for ec in range(EC):
                    pq = psum.tile([P, bpc], F32, tag="l")
                    for dc in range(DC):
                        nc.tensor.matmul(pq[:],
                                         wdec_t[dc][:, ec * P:(ec + 1) * P],
                                         dht[dc][:],
                                         start=(dc == 0), stop=(dc == DC - 1))
                    nc.vector.tensor_copy(q_sb[ec][:], pq[:])

                # W_enc: load natural, transpose into persistent w_t.
                for ec in range(EC):
                    wnat = setup.tile([P, dim], F32, tag="wnat")
                    nc.sync.dma_start(wnat[:], w_enc[ec * P:(ec + 1) * P, :])
                    for dc in range(DC):
                        ptw = psum.tile([P, P], F32, tag="tp")
                        nc.tensor.transpose(ptw[:], wnat[:, dc * P:(dc + 1) * P],
                                            identity[:])
                        nc.vector.tensor_copy(
                            w_t[dc][:, ec * P:(ec + 1) * P], ptw[:])

                # V_att chunks -> (128, 1) columns.
                v_nat = setup.tile([1, dim], F32, tag="v_nat")
                nc.sync.dma_start(v_nat[:], v_att[0:1, :])
                for ec in range(EC):
                    pv = psum.tile([P, 1], F32, tag="l")
                    nc.tensor.transpose(pv[:], v_nat[:, ec * P:(ec + 1) * P],
                                        identity[:1, :1])
                    nc.vector.tensor_copy(v_t[ec][:], pv[:])

            # ---------------- main loop ----------------
            with tc.tile_pool(name="main", bufs=2) as main:
              for _rep in range(reps):
                for b in range(bpc):
                    for it in range(NIT):
                        s0 = it * SW
                        # 1. natural loads (128 descriptors x 4KB each)
                        e_nat = []
                        for ss in range(SS):
                            t = main.tile([P, dim], F32, tag=f"enat{ss}", name=f"enat{ss}")
                            nc.sync.dma_start(
                                t[:], enc[s0 + ss * P:s0 + (ss + 1) * P, b, :])
                            e_nat.append(t)
                        # 2. transpose to (d-part, s)
                        if use_bf16_e:
                            # cast to bf16 on DVE, then XBAR DMA-transpose
                            # (128 s, 1024 d) -> (128 dp, 8 dc, 128 s)
                            e_t_all = main.tile([P, DC, SW], BF16,
                                                tag="etall", name="etall")
                            for ss in range(SS):
                                tbf = main.tile([P, dim], BF16,
                                                tag=f"ebf{ss}", name=f"ebf{ss}")
                                nc.vector.tensor_copy(tbf[:], e_nat[ss][:])
                                nc.sync.dma_start(
                                    e_t_all[:, :, ss * P:(ss + 1) * P],
                                    tbf[:], transpose=True)
                            e_t = [e_t_all[:, dc, :] for dc in range(DC)]
                            e_t_f32 = e_t
                        else:
                            e_t = []
                            for dc in range(DC):
                                pt = psum.tile([P, SW], F32, tag="tp")
                                for ss in range(SS):
                                    nc.tensor.transpose(
                                        pt[:, ss * P:(ss + 1) * P],
                                        e_nat[ss][:, dc * P:(dc + 1) * P],
                                        identity[:])
                                t = main.tile([P, SW], mm_dtype,
                                              tag=f"et{dc}", name=f"et{dc}")
                                nc.vector.tensor_copy(t[:], pt[:])
                                e_t.append(t)
                            e_t = [t[:] for t in e_t]
                            e_t_f32 = [t.bitcast(F32) for t in e_t]
                        # 3+4. phase A matmuls + tanh(.. + q)
                        sc = []
                        for ec in range(EC):
                            pk = psum.tile([P, SW], F32, tag="mm")
                            for dc in range(DC):
                                nc.tensor.matmul(
                                    pk[:],
                                    w_t[dc][:, ec * P:(ec + 1) * P],
                                    e_t[dc],
                                    start=(dc == 0), stop=(dc == DC - 1))
                            t = main.tile([P, SW], mm_dtype, tag=f"sc{ec}", name=f"sc{ec}")
                            nc.scalar.activation(t[:], pk[:], AF.Tanh,
                                                 bias=q_sb[ec][:, b:b + 1])
                            sc.append(t)
                        # 5. logits
                        pl = psum.tile([1, SW], F32, tag="l")
                        for ec in range(EC):
                            nc.tensor.matmul(pl[:], v_t[ec][:],
                                             sc[ec][:],
                                             start=(ec == 0),
                                             stop=(ec == EC - 1))
                        # 6. exp + partial denominator
                        nc.scalar.activation(
                            w_row[b][0:1, s0:s0 + SW], pl[:], AF.Exp,
                            accum_out=dpart[b][0:1, it:it + 1])
                        # 7. broadcast w across partitions via a DRAM
                        #    round-trip (0-stride DRAM-source DMA), then
                        #    accumulate context partials on DVE.
                        nc.sync.dma_start(w_scr[b:b + 1, s0:s0 + SW],
                                          w_row[b][0:1, s0:s0 + SW])
                        w_bc = main.tile([P, SW], F32, tag="w_bc")
                        nc.sync.dma_start(
                            w_bc[:],
                            w_scr[b:b + 1, s0:s0 + SW].to_broadcast((P, SW)))
                        col = b * NIT + it
                        for dc in range(DC):
                            scratch = main.tile([P, SW], F32, tag="ttr_scr")
                            nc.vector.scalar_tensor_tensor(
                                out=scratch[:], in0=e_t_f32[dc],
                                scalar=1.0, in1=w_bc[:],
                                op0=ALU.mult, op1=ALU.mult,
                                accum_out=ctx_cols[dc][:, col:col + 1])

                    # ------------ per-b epilogue ------------
                    nc.vector.tensor_reduce(den[0:1, b:b + 1], dpart[b][:],
                                            axis=AXIS.X, op=ALU.add)
                    nc.vector.reciprocal(rden[0:1, b:b + 1], den[0:1, b:b + 1])
                    # normalize alpha in place: w_row[b] *= 1/denom
                    nc.vector.tensor_scalar_mul(w_row[b][:], w_row[b][:],
                                                rden[0:1, b:b + 1])
                    nc.sync.dma_start(alpha_out[0:1, b, :], w_row[b][:])

                    ctxall = main.tile([P, DC], F32, tag="ctxall")
                    for dc in range(DC):
                        nc.vector.tensor_reduce(
                            ctxall[:, dc:dc + 1],
                            ctx_cols[dc][:, b * NIT:(b + 1) * NIT],
                            axis=AXIS.X, op=ALU.add)
                    nc.sync.dma_start(rden_scr[b:b + 1, :],
                                      rden[0:1, b:b + 1])
                    rden_b = main.tile([P, 1], F32, tag="rden_b")
                    nc.sync.dma_start(rden_b[:],
                                      rden_scr[b:b + 1, :].to_broadcast((P, 1)))
                    ctxs = main.tile([P, DC], F32, tag="ctxs")
                    nc.vector.tensor_scalar_mul(ctxs[:], ctxall[:], rden_b[:])
                    pc = psum.tile([DC, P], F32, tag="l")
                    nc.tensor.transpose(pc[:], ctxs[:], identity[:])
                    ctx_t = main.tile([DC, P], F32, tag="ctx_t")
                    nc.vector.tensor_copy(ctx_t[:], pc[:])
                    nc.sync.dma_start(ctx_out[b, 0, :], ctx_t[:])

    nc.compile()
    return nc


_NC_CACHE = {}


def _get_nc():
    key = (SEQ, BPC, DIM)
    if key not in _NC_CACHE:
        _NC_CACHE[key] = build_attention_nc()
    return _NC_CACHE[key]


def _make_in_maps(decoder_hidden, encoder_out, W_enc, W_dec, V_att):
    decoder_hidden = np.ascontiguousarray(
        np.asarray(decoder_hidden, dtype=np.float32))
    encoder_out = np.asarray(encoder_out, dtype=np.float32)
    W_enc = np.ascontiguousarray(np.asarray(W_enc, dtype=np.float32))
    W_dec = np.ascontiguousarray(np.asarray(W_dec, dtype=np.float32))
    V_att = np.ascontiguousarray(np.asarray(V_att, dtype=np.float32))
    in_maps = []
    for c in range(N_CORES):
        bs = slice(c * BPC, (c + 1) * BPC)
        in_maps.append({
            "encoder_out": np.ascontiguousarray(encoder_out[:, bs, :]),
            "decoder_hidden": decoder_hidden[bs],
            "W_enc": W_enc,
            "W_dec": W_dec,
            "V_att": V_att,
        })
    return in_maps


def run_on_hw(decoder_hidden, encoder_out, W_enc, W_dec, V_att, **bench_kwargs):
    """Run on the 8 NeuronCores; returns (BassKernelResults, context, alpha)."""
    from concourse.bass_utils import run_bass_kernel_spmd

    nc = _get_nc()
    in_maps = _make_in_maps(decoder_hidden, encoder_out, W_enc, W_dec, V_att)
    res = run_bass_kernel_spmd(nc, in_maps, list(range(N_CORES)),
                               **bench_kwargs)
    context = np.concatenate([res.results[c]["context"]
                              for c in range(N_CORES)], axis=0)
    alpha = np.concatenate([res.results[c]["alpha"]
                            for c in range(N_CORES)], axis=1)
    return res, context, alpha


def kernel(decoder_hidden, encoder_out, W_enc, W_dec, V_att):
    _, context, alpha = run_on_hw(decoder_hidden, encoder_out, W_enc, W_dec,
                                  V_att)
    return context, alpha
